# revision 20
# baseline (speedup 1.0000x reference)
"""ContrastiveLoss (cosine-similarity based) on 8 Trainium2 NeuronCores.

Data-parallel: batch B=8192 is sharded 1024 rows/core across 8 cores.
Inputs are cast to bf16 on host (halves DMA traffic; all accumulation is
f32 — the scalar mean's rel err vs the f32 reference is ~1e-6).
Per core, 8 row-tiles of [128 rows x 4096], triple-buffered DMA.

variant "base" (default, best measured ~69us/pass):
  - DVE: tensor_mul (bf16 2x mode) + reduce_sum -> per-row dot
  - ACT: two Square activations w/ accum_out -> |a|^2, |b|^2
Measured HW facts that killed the alternatives (all verified on-device):
  - accum_out ops run at 1x (1.04 ns/col) regardless of dtype; plain
    elementwise stt runs ~6x faster (657ns/[128x4096]) -- but every
    chained instruction whose semaphore isn't pre-satisfied at decode
    costs ~1us, so halving-add-tree reductions (11 ops/tile, "tree"
    169us; pair-interleaved "tree2" 109us) lose to 4 big accum ops.
  - Pool rejects TensorScalarPtr ("fused3" fails compile); sliced
    accum splits ("fused2") measured 75us -- the per-op overhead eats
    the column-split savings.
DMA floor (bf16, 16MB/core) ~37us; compute plateau ~69us is ACT/DVE
instruction-serialization bound, not bandwidth bound.

Raw Bass (explicit semaphores) because this compiler build rejects
Tile-generated multi-wait instructions.
Tiny [128,8] epilogue computes per-row losses, reduced to [128,1]/core.
Host sums the 8x128 partials and divides by B (mean).
"""

import sys

import numpy as np

if "/opt/trn_rl_repo" not in sys.path:
    sys.path.append("/opt/trn_rl_repo")

B, D = 8192, 4096
NCORES = 8
BS = B // NCORES  # rows per core
P = 128  # SBUF partitions
RT = BS // P  # row-tiles per core
NBUF = 2  # input double-buffering (best measured config)
EPS = 1e-9
MARGIN = 1.0
PC = 3502  # sqb columns handled by Pool (rest by ACT)

_CACHE: dict = {}
LAST_EXEC_TIME_NS = None
TRACE = False


def _build_nc(reps=1, in_dt="bfloat16", variant="fused3", nbuf=NBUF, pc=PC):
    """Build the kernel. reps>1 repeats the main loop (re-reading the same
    DRAM) for on-device steady-state timing."""
    import concourse.bass as bass
    import concourse.mybir as mybir

    f32 = mybir.dt.float32
    idt = getattr(mybir.dt, in_dt)
    AF = mybir.ActivationFunctionType
    ALU = mybir.AluOpType
    X = mybir.AxisListType.X

    nc = bass.Bass()
    o1 = nc.declare_dram_parameter("output1", [BS, D], idt, isOutput=False)
    o2 = nc.declare_dram_parameter("output2", [BS, D], idt, isOutput=False)
    tgt = nc.declare_dram_parameter("target_f32", [P, RT], f32, isOutput=False)
    out = nc.declare_dram_parameter("out", [P, 1], f32, isOutput=True)

    t_sem = nc.alloc_semaphore("t_sem")
    a_sems = [nc.alloc_semaphore(f"a{i}_sem") for i in range(nbuf)]
    b_sems = [nc.alloc_semaphore(f"b{i}_sem") for i in range(nbuf)]
    out_sem = nc.alloc_semaphore("out_sem")
    v_sem = nc.alloc_semaphore("v_sem")  # DVE progress
    s_sem = nc.alloc_semaphore("s_sem")  # ACT progress
    g_sem = nc.alloc_semaphore("g_sem")  # Pool progress

    from contextlib import ExitStack

    with ExitStack() as ctx:

        def sb(shape, name, dt=f32):
            return ctx.enter_context(nc.sbuf_tensor(name, shape, dt))

        a_bufs = [sb([P, D], f"abuf{i}", idt) for i in range(nbuf)]
        b_bufs = [sb([P, D], f"bbuf{i}", idt) for i in range(nbuf)]
        sd = sb([P, D], "sd", idt)   # DVE stt product out (unread)
        sp = sb([P, D], "sp", idt)   # Pool stt square out (unread)
        sa = sb([P, D], "sa", idt)   # ACT square out (unread)
        if variant in ("tree", "tree2"):
            w_bufs = [sb([P, 2, D], f"wbuf{i}", idt) for i in range(2)]
            accT = sb([P, 2, RT], "accT")  # [prod; sqb] per-tile sums
        num = sb([P, RT], "num")
        n1 = sb([P, RT], "n1")
        n2a = sb([P, RT], "n2a")  # ACT part of |b|^2 (tail cols)
        n2p = sb([P, RT], "n2p")  # Pool part of |b|^2 (head cols)
        t_tile = sb([P, RT], "t_tile")
        e_n2 = sb([P, RT], "e_n2")
        e_d2 = sb([P, RT], "e_d2")
        e_den = sb([P, RT], "e_den")
        e_inv = sb([P, RT], "e_inv")
        e_cos = sb([P, RT], "e_cos")
        e_dist = sb([P, RT], "e_dist")
        e_de = sb([P, RT], "e_de")
        e_s = sb([P, RT], "e_s")
        e_h = sb([P, RT], "e_h")
        e_h2 = sb([P, RT], "e_h2")
        e_dmh = sb([P, RT], "e_dmh")
        e_tdm = sb([P, RT], "e_tdm")
        e_li = sb([P, RT], "e_li")
        red = sb([P, 1], "red")
        block = ctx.enter_context(nc.Block())

        NT = reps * RT  # total tiles processed
        # fused2p: Pool squares QP cols of b and folds them with an in-place
        # add-tree (1 mul + log2(QP) adds); DVE takes SD2 more sqb cols,
        # ACT the rest.
        QP = 1024
        NPOOL_TILE = 1 + QP.bit_length() - 1  # mul + 10 adds
        # per-tile op counts per engine
        if variant == "fused3":
            NV_TILE, NS_TILE, NG_TILE = 1, 2, 1
            N_EPI_V = 11
        elif variant == "fused2":
            NV_TILE, NS_TILE, NG_TILE = 2, 2, 0
            N_EPI_V = 11
        elif variant == "fused2p":
            NV_TILE, NS_TILE, NG_TILE = 2, 2, NPOOL_TILE
            N_EPI_V = 12
        elif variant == "base":
            NV_TILE, NS_TILE, NG_TILE = 2, 2, 0
            N_EPI_V = 10
        elif variant in ("tree", "tree2"):
            # DVE: prod + sqb elementwise (fast 4x stt), then fold both
            # regions together with an in-place halving add-tree (3D APs),
            # final tiny reduce -> accT[:, :, j]. ACT: one Square+accum (n1).
            # tree2 interleaves the DVE streams of tile pairs so every
            # intra-tile dependency is >=2 stream positions back and the
            # engine never stalls on just-produced semaphores.
            NV_TILE, NS_TILE, NG_TILE = 11, 1, 0
            N_EPI_V = 10
        else:
            raise ValueError(variant)
        SD2 = pc if variant in ("fused2", "fused2p") else 1609
        NV_LOOP = NV_TILE * NT
        NS_LOOP = NS_TILE * NT
        NG_LOOP = NG_TILE * NT
        V_TOTAL = NV_LOOP + N_EPI_V
        S_TOTAL = NS_LOOP + 3

        @block.sync
        def _(sync):
            sync.dma_start(out=t_tile[:], in_=tgt[:]).then_inc(t_sem, 16)
            for g in range(NT):
                j = g % RT  # row-block within the shard
                k, r = g % nbuf, g // nbuf  # buffer index, reload round
                if g >= nbuf:
                    # recycle buffer k: all consumers done with tile g-nbuf,
                    # and the previous DMA into this buffer fully completed
                    gp = g - nbuf
                    if variant == "tree":
                        # only d1/d2 (prod, sqb) read a/b; folds don't
                        sync.wait_ge(v_sem, NV_TILE * gp + 2)
                    elif variant == "tree2":
                        # pair of tile gp fully done
                        sync.wait_ge(v_sem, 2 * NV_TILE * (gp // 2 + 1))
                    else:
                        sync.wait_ge(v_sem, NV_TILE * gp + NV_TILE)
                    sync.wait_ge(s_sem, NS_TILE * gp + NS_TILE)
                    if NG_TILE:
                        sync.wait_ge(g_sem, NG_TILE * gp + NG_TILE)
                    sync.wait_ge(a_sems[k], 16 * r)
                    sync.wait_ge(b_sems[k], 16 * r)
                sync.dma_start(
                    out=a_bufs[k][:], in_=o1[j * P : (j + 1) * P, :]
                ).then_inc(a_sems[k], 16)
                sync.dma_start(
                    out=b_bufs[k][:], in_=o2[j * P : (j + 1) * P, :]
                ).then_inc(b_sems[k], 16)
            # epilogue result
            sync.wait_ge(v_sem, V_TOTAL)
            sync.dma_start(out=out[:], in_=red[:]).then_inc(out_sem, 16)
            sync.wait_ge(out_sem, 16)

        if variant == "fused3":

            @block.gpsimd
            def _(gpsimd):
                for g in range(NT):
                    j = g % RT
                    k, r = g % nbuf, g // nbuf
                    gpsimd.wait_ge(b_sems[k], 16 * (r + 1))
                    if g:
                        gpsimd.wait_ge(g_sem, g)
                    nc.gpsimd.scalar_tensor_tensor(
                        sp[:, 0:pc],
                        b_bufs[k][:, 0:pc],
                        1.0,
                        b_bufs[k][:, 0:pc],
                        op0=ALU.mult,
                        op1=ALU.mult,
                        accum_out=n2p[:, j : j + 1],
                    ).then_inc(g_sem, 1)

        @block.vector
        def _(vector):
            vi = 0

            def vop(inst):
                nonlocal vi
                vi += 1
                return inst.then_inc(v_sem, 1)

            def vwait(idx):
                vector.wait_ge(v_sem, idx)

            if variant == "tree2":
                # ops of a pair (g0, g0+1), phase-interleaved
                for q in range(NT // 2):
                    pair = (2 * q, 2 * q + 1)
                    for g in pair:
                        k, r = g % nbuf, g // nbuf
                        vector.wait_ge(a_sems[k], 16 * (r + 1))
                        vector.wait_ge(b_sems[k], 16 * (r + 1))
                    base_vi = vi

                    def ph_wait(g_idx, phase):
                        # wait for (phase-1) op of same tile in this pair
                        if phase == 0:
                            if base_vi:
                                vwait(base_vi)
                        else:
                            vwait(base_vi + 2 * (phase - 1) + g_idx + 1)

                    for phase in range(11):
                        for gi, g in enumerate(pair):
                            j = g % RT
                            k = g % nbuf
                            wb = w_bufs[g % 2]
                            ph_wait(gi, phase)
                            if phase == 0:
                                vop(nc.vector.scalar_tensor_tensor(
                                    wb[:, 0, :], a_bufs[k][:], 1.0,
                                    b_bufs[k][:], op0=ALU.mult, op1=ALU.mult))
                            elif phase == 1:
                                vop(nc.vector.scalar_tensor_tensor(
                                    wb[:, 1, :], b_bufs[k][:], 1.0,
                                    b_bufs[k][:], op0=ALU.mult, op1=ALU.mult))
                            elif phase <= 9:
                                w = D >> phase  # 2048 .. 16
                                vop(nc.vector.scalar_tensor_tensor(
                                    wb[:, :, 0:w], wb[:, :, 0:w], 1.0,
                                    wb[:, :, w : 2 * w],
                                    op0=ALU.mult, op1=ALU.add))
                            else:
                                vop(nc.vector.reduce_sum(
                                    accT[:, :, j : j + 1], wb[:, :, 0:16],
                                    axis=X))
            for g in range(NT if variant != "tree2" else 0):
                j = g % RT
                k, r = g % nbuf, g // nbuf
                if variant == "tree":
                    wb = w_bufs[g % 2]
                    vector.wait_ge(a_sems[k], 16 * (r + 1))
                    vector.wait_ge(b_sems[k], 16 * (r + 1))
                    if vi:
                        vwait(vi)
                    vop(
                        nc.vector.scalar_tensor_tensor(
                            wb[:, 0, :], a_bufs[k][:], 1.0, b_bufs[k][:],
                            op0=ALU.mult, op1=ALU.mult,
                        )
                    )
                    vwait(vi)
                    vop(
                        nc.vector.scalar_tensor_tensor(
                            wb[:, 1, :], b_bufs[k][:], 1.0, b_bufs[k][:],
                            op0=ALU.mult, op1=ALU.mult,
                        )
                    )
                    w = D // 2
                    while w >= 16:
                        vwait(vi)
                        vop(
                            nc.vector.scalar_tensor_tensor(
                                wb[:, :, 0:w], wb[:, :, 0:w], 1.0,
                                wb[:, :, w : 2 * w],
                                op0=ALU.mult, op1=ALU.add,
                            )
                        )
                        w //= 2
                    vwait(vi)
                    vop(
                        nc.vector.reduce_sum(
                            accT[:, :, j : j + 1], wb[:, :, 0:16], axis=X
                        )
                    )
                    continue
                vector.wait_ge(a_sems[k], 16 * (r + 1))
                vector.wait_ge(b_sems[k], 16 * (r + 1))
                if vi:
                    vwait(vi)
                if variant == "base":
                    vop(nc.vector.tensor_mul(sd[:], a_bufs[k][:], b_bufs[k][:]))
                    vwait(vi)
                    vop(nc.vector.reduce_sum(num[:, j : j + 1], sd[:], axis=X))
                else:
                    vop(
                        nc.vector.scalar_tensor_tensor(
                            sd[:],
                            a_bufs[k][:],
                            1.0,
                            b_bufs[k][:],
                            op0=ALU.mult,
                            op1=ALU.mult,
                            accum_out=num[:, j : j + 1],
                        )
                    )
                    if variant == "fused2":
                        vwait(vi)
                        vop(
                            nc.vector.scalar_tensor_tensor(
                                sd[:, 0:SD2],
                                b_bufs[k][:, 0:SD2],
                                1.0,
                                b_bufs[k][:, 0:SD2],
                                op0=ALU.mult,
                                op1=ALU.mult,
                                accum_out=n2p[:, j : j + 1],
                            )
                        )
            # ---- epilogue ----
            vector.wait_ge(s_sem, NS_LOOP)  # all n1/n2a ready
            if variant == "base":
                vwait(vi)
                vop(nc.vector.tensor_mul(e_d2[:], n1[:], n2a[:]))
            elif variant in ("tree", "tree2"):
                vwait(vi)
                vop(nc.vector.tensor_mul(e_d2[:], n1[:], accT[:, 1, :]))
            else:
                if variant == "fused3":
                    vector.wait_ge(g_sem, NG_LOOP)  # n2p ready
                vwait(vi)
                vop(nc.vector.tensor_add(e_n2[:], n2a[:], n2p[:]))
                vwait(vi)
                vop(nc.vector.tensor_mul(e_d2[:], n1[:], e_n2[:]))
            vector.wait_ge(s_sem, NS_LOOP + 1)  # den ready
            vwait(vi)
            vop(nc.vector.reciprocal(e_inv[:], e_den[:]))
            vwait(vi)
            num_ap = accT[:, 0, :] if variant in ("tree", "tree2") else num[:]
            vop(nc.vector.tensor_mul(e_cos[:], num_ap, e_inv[:]))
            # dist = 0.5 - 0.5*cos ; de = dist + eps
            vwait(vi)
            vop(
                nc.vector.tensor_scalar(
                    e_dist[:], e_cos[:], -0.5, 0.5, ALU.mult, ALU.add
                )
            )
            vwait(vi)
            vop(nc.vector.tensor_scalar_add(e_de[:], e_dist[:], EPS))
            vector.wait_ge(s_sem, S_TOTAL)  # h ready
            vwait(vi)
            vop(nc.vector.tensor_mul(e_h2[:], e_h[:], e_h[:]))
            vwait(vi)
            vop(nc.vector.tensor_sub(e_dmh[:], e_dist[:], e_h2[:]))
            vector.wait_ge(t_sem, 16)  # t_tile loaded
            vwait(vi)
            vop(nc.vector.tensor_mul(e_tdm[:], t_tile[:], e_dmh[:]))
            vwait(vi)
            vop(nc.vector.tensor_add(e_li[:], e_tdm[:], e_h2[:]))
            vwait(vi)
            vop(nc.vector.reduce_sum(red[:], e_li[:], axis=X))
            assert vi == V_TOTAL, (vi, V_TOTAL)

        def _scalar_body(scalar):
            si = 0

            def sop(inst):
                nonlocal si
                si += 1
                return inst.then_inc(s_sem, 1)

            def swait(idx):
                scalar.wait_ge(s_sem, idx)

            for g in range(NT):
                j = g % RT
                k, r = g % nbuf, g // nbuf
                scalar.wait_ge(a_sems[k], 16 * (r + 1))
                if si:
                    swait(si)
                sop(
                    nc.scalar.activation(
                        sa[:], a_bufs[k][:], AF.Square,
                        accum_out=n1[:, j : j + 1],
                    )
                )
                if variant in ("tree", "tree2"):
                    continue
                scalar.wait_ge(b_sems[k], 16 * (r + 1))
                swait(si)
                if variant == "fused3":
                    sop(
                        nc.scalar.activation(
                            sa[:, pc:D], b_bufs[k][:, pc:D], AF.Square,
                            accum_out=n2a[:, j : j + 1],
                        )
                    )
                elif variant == "fused2":
                    sop(
                        nc.scalar.activation(
                            sa[:, SD2:D], b_bufs[k][:, SD2:D], AF.Square,
                            accum_out=n2a[:, j : j + 1],
                        )
                    )
                else:  # base: full |b|^2 on ACT
                    sop(
                        nc.scalar.activation(
                            sa[:], b_bufs[k][:], AF.Square,
                            accum_out=n2a[:, j : j + 1],
                        )
                    )
            # ---- epilogue ----
            ep_off = 1 if variant in ("base", "tree", "tree2") else 2
            scalar.wait_ge(v_sem, NV_LOOP + ep_off)  # d2 ready
            swait(si)
            sop(nc.scalar.activation(e_den[:], e_d2[:], AF.Sqrt))
            scalar.wait_ge(v_sem, NV_LOOP + ep_off + 4)  # de ready
            swait(si)
            sop(nc.scalar.activation(e_s[:], e_de[:], AF.Sqrt))
            swait(si)
            sop(
                nc.scalar.activation(
                    e_h[:], e_s[:], AF.Relu, bias=MARGIN, scale=-1.0
                )
            )
            assert si == S_TOTAL, (si, S_TOTAL)

        block.scalar(_scalar_body)

    nc.all_engine_barrier()
    nc.clear_and_free_semaphores(
        [t_sem, *a_sems, *b_sems, out_sem, v_sem, s_sem, g_sem]
    )
    nc.all_engine_barrier()
    return nc


VARIANT = "base"
IN_DT = "bfloat16"  # input dtype on device; "float32" for full precision


def get_nc_variant(reps, in_dt, variant, nbuf=NBUF, pc=PC):
    key = ("nc", reps, in_dt, variant, nbuf, pc)
    if key not in _CACHE:
        _CACHE[key] = _build_nc(reps, in_dt, variant, nbuf, pc)
    return _CACHE[key]


def get_nc(reps=1, in_dt=None, variant=None):
    return get_nc_variant(reps, in_dt or IN_DT, variant or VARIANT)


def _np_in_dt(in_dt):
    if in_dt == "float32":
        return np.float32
    import ml_dtypes

    return getattr(ml_dtypes, in_dt)


def make_in_maps(output1, output2, target, in_dt=None):
    in_dt = in_dt or IN_DT
    npdt = _np_in_dt(in_dt)
    o1 = np.ascontiguousarray(np.asarray(output1).astype(npdt))
    o2 = np.ascontiguousarray(np.asarray(output2).astype(npdt))
    t = np.asarray(target).astype(np.float32)
    in_maps = []
    for c in range(NCORES):
        sl = slice(c * BS, (c + 1) * BS)
        # t_tile[p, j] = t_core[j*128 + p]
        tcore = np.ascontiguousarray(t[sl].reshape(RT, P).T)
        in_maps.append(
            {
                "output1": np.ascontiguousarray(o1[sl]),
                "output2": np.ascontiguousarray(o2[sl]),
                "target_f32": tcore,
            }
        )
    return in_maps


def kernel(output1, output2, target):
    global LAST_EXEC_TIME_NS
    from concourse.bass_utils import run_bass_kernel_spmd

    nc = get_nc()
    in_maps = make_in_maps(output1, output2, target)
    res = run_bass_kernel_spmd(
        nc, in_maps, core_ids=list(range(NCORES)), trace=TRACE
    )
    LAST_EXEC_TIME_NS = res.exec_time_ns
    total = np.float64(0.0)
    for r in res.results:
        total += r["out"].astype(np.float64).sum()
    mean = 0.5 * total / B
    return np.array(mean, dtype=np.float32)


def _reduce_results(out_shards):
    total = np.float64(0.0)
    for r in out_shards:
        total += np.asarray(r, dtype=np.float64).sum()
    return np.array(0.5 * total / B, dtype=np.float32)


def _make_executable(nc):
    """Replicate run_bass_via_pjrt's sharded executable, returning
    (fn, dev_in_builder, out_avals, n_params). The hook requires the HLO to
    be exactly the bass_exec custom call, so no loops are possible."""
    import jax
    from jax.experimental.shard_map import shard_map
    from jax.sharding import Mesh, NamedSharding, PartitionSpec

    from concourse import mybir
    from concourse.bass2jax import (
        _bass_exec_p,
        install_neuronx_cc_hook,
        partition_id_tensor,
    )

    install_neuronx_cc_hook()
    partition_name = nc.partition_id_tensor.name if nc.partition_id_tensor else None
    in_names, out_names, out_avals, zero_outs = [], [], [], []
    for alloc in nc.m.functions[0].allocations:
        if not isinstance(alloc, mybir.MemoryLocationSet):
            continue
        name = alloc.memorylocations[0].name
        if alloc.kind == "ExternalInput":
            if name != partition_name:
                in_names.append(name)
        elif alloc.kind == "ExternalOutput":
            shape = tuple(alloc.tensor_shape)
            dtype = mybir.dt.np(alloc.dtype)
            out_names.append(name)
            out_avals.append(jax.core.ShapedArray(shape, dtype))
            zero_outs.append(np.zeros(shape, dtype))
    n_params = len(in_names)
    all_names = tuple(
        in_names + out_names + ([partition_name] if partition_name else [])
    )

    def _body(*args):
        operands = list(args)
        operands.append(partition_id_tensor())
        outs = _bass_exec_p.bind(
            *operands,
            out_avals=tuple(out_avals),
            in_names=all_names,
            out_names=tuple(out_names),
            lowering_input_output_aliases=(),
            sim_require_finite=True,
            sim_require_nnan=True,
            nc=nc,
        )
        return tuple(outs)

    devices = jax.devices()[:NCORES]
    mesh = Mesh(np.asarray(devices), ("core",))
    in_specs = (PartitionSpec("core"),) * (n_params + 1)
    out_specs = (PartitionSpec("core"),) * len(out_names)
    fn = jax.jit(
        shard_map(
            _body, mesh=mesh, in_specs=in_specs, out_specs=out_specs,
            check_rep=False,
        ),
        keep_unused=True,
    )
    sharding = NamedSharding(mesh, PartitionSpec("core"))
    return fn, sharding, in_names, out_avals, zero_outs, n_params


def benchmark(output1, output2, target, reps=96, dispatches=(4, 24), nc=None):
    """Measure steady-state device time per full pass over the data.

    The axon relay has ~50-100ms of noisy per-dispatch overhead, so a
    single execution can't be timed. Instead: build a kernel that loops
    the pipeline `reps` times on-device (re-reading the same DRAM), then
    time K back-to-back dispatches for two values of K. The slope is the
    device time per dispatch (~reps passes), immune to the constant
    overhead; divide by reps for per-pass time.
    Returns (result, per_pass_ns, info)."""
    import time

    import jax

    in_maps = make_in_maps(output1, output2, target)
    info = {}

    if nc is None:
        nc = get_nc(reps)
    fn, sharding, in_names, out_avals, zero_outs, n_params = _make_executable(nc)
    per_core = [[np.asarray(m[name]) for name in in_names] for m in in_maps]
    concat_in = [
        np.concatenate([per_core[c][i] for c in range(NCORES)], axis=0)
        for i in range(n_params)
    ]
    dev_in = [jax.device_put(x, sharding) for x in concat_in]
    concat_zero = np.zeros(
        (NCORES * zero_outs[0].shape[0], *zero_outs[0].shape[1:]),
        zero_outs[0].dtype,
    )
    dev_zero = jax.device_put(concat_zero, sharding)

    out = fn(*dev_in, dev_zero)[0]
    out.block_until_ready()  # compile + warmup
    result_arr = np.asarray(out).reshape(NCORES, *out_avals[0].shape)
    result = _reduce_results([result_arr[c] for c in range(NCORES)])

    def timed(k, tries=5):
        best = None
        for _ in range(tries):
            t0 = time.perf_counter()
            last = None
            for _ in range(k):
                last = fn(*dev_in, dev_zero)[0]
            last.block_until_ready()
            dt = time.perf_counter() - t0
            best = dt if best is None else min(best, dt)
        return best

    k1, k2 = dispatches
    # interleave the two dispatch counts so slow-device drift hits both
    # equally instead of biasing the slope
    t1a, t2a = timed(k1, 3), timed(k2, 3)
    t1b, t2b = timed(k1, 3), timed(k2, 3)
    t1, t2 = min(t1a, t1b), min(t2a, t2b)
    per_pass_ns = (t2 - t1) / (k2 - k1) / reps * 1e9
    info["dispatch_times_ms"] = {k1: t1 * 1e3, k2: t2 * 1e3}
    info["reps"] = reps
    _CACHE["last_info"] = info
    return result, per_pass_ns, info


# revision 22
# speedup vs baseline: 1.0361x; 1.0361x over previous
"""ContrastiveLoss (cosine-similarity based) on 8 Trainium2 NeuronCores.

Data-parallel: batch B=8192 is sharded 1024 rows/core across 8 cores.
Inputs are cast to bf16 on host (halves DMA traffic; all accumulation is
f32 — the scalar mean's rel err vs the f32 reference is ~1e-6).
Per core, 8 row-tiles of [128 rows x 4096], triple-buffered DMA.

variant "base" (default, best measured ~69us/pass):
  - DVE: tensor_mul (bf16 2x mode) + reduce_sum -> per-row dot
  - ACT: two Square activations w/ accum_out -> |a|^2, |b|^2
Measured HW facts that killed the alternatives (all verified on-device):
  - accum_out ops run at 1x (1.04 ns/col) regardless of dtype; plain
    elementwise stt runs ~6x faster (657ns/[128x4096]) -- but every
    chained instruction whose semaphore isn't pre-satisfied at decode
    costs ~1us, so halving-add-tree reductions (11 ops/tile, "tree"
    169us; pair-interleaved "tree2" 109us) lose to 4 big accum ops.
  - Pool rejects TensorScalarPtr ("fused3" fails compile); sliced
    accum splits ("fused2") measured 75us -- the per-op overhead eats
    the column-split savings.
DMA floor (bf16, 16MB/core) ~37us; compute plateau ~69us is ACT/DVE
instruction-serialization bound, not bandwidth bound.

Raw Bass (explicit semaphores) because this compiler build rejects
Tile-generated multi-wait instructions.
Tiny [128,8] epilogue computes per-row losses, reduced to [128,1]/core.
Host sums the 8x128 partials and divides by B (mean).
"""

import sys

import numpy as np

if "/opt/trn_rl_repo" not in sys.path:
    sys.path.append("/opt/trn_rl_repo")

B, D = 8192, 4096
NCORES = 8
BS = B // NCORES  # rows per core
P = 128  # SBUF partitions
RT = BS // P  # row-tiles per core
NBUF = 4  # 4 bufs so a pair's DMAs overlap the previous pair
EPS = 1e-9
MARGIN = 1.0
PC = 3502  # sqb columns handled by Pool (rest by ACT)

_CACHE: dict = {}
LAST_EXEC_TIME_NS = None
TRACE = False


def _build_nc(reps=1, in_dt="bfloat16", variant="fused3", nbuf=NBUF, pc=PC):
    """Build the kernel. reps>1 repeats the main loop (re-reading the same
    DRAM) for on-device steady-state timing."""
    import concourse.bass as bass
    import concourse.mybir as mybir

    f32 = mybir.dt.float32
    idt = getattr(mybir.dt, in_dt)
    AF = mybir.ActivationFunctionType
    ALU = mybir.AluOpType
    X = mybir.AxisListType.X

    nc = bass.Bass()
    o1 = nc.declare_dram_parameter("output1", [BS, D], idt, isOutput=False)
    o2 = nc.declare_dram_parameter("output2", [BS, D], idt, isOutput=False)
    tgt = nc.declare_dram_parameter("target_f32", [P, RT], f32, isOutput=False)
    out = nc.declare_dram_parameter("out", [P, 1], f32, isOutput=True)

    t_sem = nc.alloc_semaphore("t_sem")
    a_sems = [nc.alloc_semaphore(f"a{i}_sem") for i in range(nbuf)]
    b_sems = [nc.alloc_semaphore(f"b{i}_sem") for i in range(nbuf)]
    out_sem = nc.alloc_semaphore("out_sem")
    v_sem = nc.alloc_semaphore("v_sem")  # DVE progress
    s_sem = nc.alloc_semaphore("s_sem")  # ACT progress
    g_sem = nc.alloc_semaphore("g_sem")  # Pool progress

    from contextlib import ExitStack

    with ExitStack() as ctx:

        def sb(shape, name, dt=f32):
            return ctx.enter_context(nc.sbuf_tensor(name, shape, dt))

        a_bufs = [sb([P, D], f"abuf{i}", idt) for i in range(nbuf)]
        b_bufs = [sb([P, D], f"bbuf{i}", idt) for i in range(nbuf)]
        sd = sb([P, D], "sd", idt)   # DVE stt product out (unread)
        sp = sb([P, D], "sp", idt)   # Pool stt square out (unread)
        sa = sb([P, D], "sa", idt)   # ACT square out (unread)
        if variant in ("tree", "tree2"):
            w_bufs = [sb([P, 2, D], f"wbuf{i}", idt) for i in range(2)]
            accT = sb([P, 2, RT], "accT")  # [prod; sqb] per-tile sums
        num = sb([P, RT], "num")
        n1 = sb([P, RT], "n1")
        n2a = sb([P, RT], "n2a")  # ACT part of |b|^2 (tail cols)
        n2p = sb([P, RT], "n2p")  # Pool part of |b|^2 (head cols)
        t_tile = sb([P, RT], "t_tile")
        e_n2 = sb([P, RT], "e_n2")
        e_d2 = sb([P, RT], "e_d2")
        e_den = sb([P, RT], "e_den")
        e_inv = sb([P, RT], "e_inv")
        e_cos = sb([P, RT], "e_cos")
        e_dist = sb([P, RT], "e_dist")
        e_de = sb([P, RT], "e_de")
        e_s = sb([P, RT], "e_s")
        e_h = sb([P, RT], "e_h")
        e_h2 = sb([P, RT], "e_h2")
        e_dmh = sb([P, RT], "e_dmh")
        e_tdm = sb([P, RT], "e_tdm")
        e_li = sb([P, RT], "e_li")
        red = sb([P, 1], "red")
        block = ctx.enter_context(nc.Block())

        NT = reps * RT  # total tiles processed
        # fused2p: Pool squares QP cols of b and folds them with an in-place
        # add-tree (1 mul + log2(QP) adds); DVE takes SD2 more sqb cols,
        # ACT the rest.
        QP = 1024
        NPOOL_TILE = 1 + QP.bit_length() - 1  # mul + 10 adds
        # per-tile op counts per engine
        if variant == "fused3":
            NV_TILE, NS_TILE, NG_TILE = 1, 2, 1
            N_EPI_V = 11
        elif variant == "fused2":
            NV_TILE, NS_TILE, NG_TILE = 2, 2, 0
            N_EPI_V = 11
        elif variant == "fused2p":
            NV_TILE, NS_TILE, NG_TILE = 2, 2, NPOOL_TILE
            N_EPI_V = 12
        elif variant in ("base", "basep"):
            # basep: DVE stream pair-interleaved (mul g, mul g+1, reduce g,
            # reduce g+1; sd/sp ping-pong) so each op's dependency is 2
            # stream positions back and pre-satisfied at decode.
            NV_TILE, NS_TILE, NG_TILE = 2, 2, 0
            N_EPI_V = 10
        elif variant in ("tree", "tree2"):
            # DVE: prod + sqb elementwise (fast 4x stt), then fold both
            # regions together with an in-place halving add-tree (3D APs),
            # final tiny reduce -> accT[:, :, j]. ACT: one Square+accum (n1).
            # tree2 interleaves the DVE streams of tile pairs so every
            # intra-tile dependency is >=2 stream positions back and the
            # engine never stalls on just-produced semaphores.
            NV_TILE, NS_TILE, NG_TILE = 11, 1, 0
            N_EPI_V = 10
        else:
            raise ValueError(variant)
        SD2 = pc if variant in ("fused2", "fused2p") else 1609
        NV_LOOP = NV_TILE * NT
        NS_LOOP = NS_TILE * NT
        NG_LOOP = NG_TILE * NT
        V_TOTAL = NV_LOOP + N_EPI_V
        S_TOTAL = NS_LOOP + 3

        @block.sync
        def _(sync):
            sync.dma_start(out=t_tile[:], in_=tgt[:]).then_inc(t_sem, 16)
            for g in range(NT):
                j = g % RT  # row-block within the shard
                k, r = g % nbuf, g // nbuf  # buffer index, reload round
                if g >= nbuf:
                    # recycle buffer k: all consumers done with tile g-nbuf,
                    # and the previous DMA into this buffer fully completed
                    gp = g - nbuf
                    if variant == "tree":
                        # only d1/d2 (prod, sqb) read a/b; folds don't
                        sync.wait_ge(v_sem, NV_TILE * gp + 2)
                    elif variant in ("tree2", "basep"):
                        # pair of tile gp fully done
                        sync.wait_ge(v_sem, 2 * NV_TILE * (gp // 2 + 1))
                    else:
                        sync.wait_ge(v_sem, NV_TILE * gp + NV_TILE)
                    sync.wait_ge(s_sem, NS_TILE * gp + NS_TILE)
                    if NG_TILE:
                        sync.wait_ge(g_sem, NG_TILE * gp + NG_TILE)
                    sync.wait_ge(a_sems[k], 16 * r)
                    sync.wait_ge(b_sems[k], 16 * r)
                sync.dma_start(
                    out=a_bufs[k][:], in_=o1[j * P : (j + 1) * P, :]
                ).then_inc(a_sems[k], 16)
                sync.dma_start(
                    out=b_bufs[k][:], in_=o2[j * P : (j + 1) * P, :]
                ).then_inc(b_sems[k], 16)
            # epilogue result
            sync.wait_ge(v_sem, V_TOTAL)
            sync.dma_start(out=out[:], in_=red[:]).then_inc(out_sem, 16)
            sync.wait_ge(out_sem, 16)

        if variant == "fused3":

            @block.gpsimd
            def _(gpsimd):
                for g in range(NT):
                    j = g % RT
                    k, r = g % nbuf, g // nbuf
                    gpsimd.wait_ge(b_sems[k], 16 * (r + 1))
                    if g:
                        gpsimd.wait_ge(g_sem, g)
                    nc.gpsimd.scalar_tensor_tensor(
                        sp[:, 0:pc],
                        b_bufs[k][:, 0:pc],
                        1.0,
                        b_bufs[k][:, 0:pc],
                        op0=ALU.mult,
                        op1=ALU.mult,
                        accum_out=n2p[:, j : j + 1],
                    ).then_inc(g_sem, 1)

        @block.vector
        def _(vector):
            vi = 0

            def vop(inst):
                nonlocal vi
                vi += 1
                return inst.then_inc(v_sem, 1)

            def vwait(idx):
                vector.wait_ge(v_sem, idx)

            if variant == "basep":
                sd_pair = [sd, sp]
                for q in range(NT // 2):
                    pair = (2 * q, 2 * q + 1)
                    for g in pair:
                        k, r = g % nbuf, g // nbuf
                        vector.wait_ge(a_sems[k], 16 * (r + 1))
                        vector.wait_ge(b_sems[k], 16 * (r + 1))
                    base_vi = vi
                    for gi, g in enumerate(pair):
                        k = g % nbuf
                        if base_vi:
                            vwait(base_vi)  # prior pair fully done (WAW sd)
                        vop(nc.vector.tensor_mul(
                            sd_pair[gi][:], a_bufs[k][:], b_bufs[k][:]))
                    for gi, g in enumerate(pair):
                        j = g % RT
                        vwait(base_vi + gi + 1)  # mul of this tile done
                        vop(nc.vector.reduce_sum(
                            num[:, j : j + 1], sd_pair[gi][:], axis=X))
            if variant == "tree2":
                # ops of a pair (g0, g0+1), phase-interleaved
                for q in range(NT // 2):
                    pair = (2 * q, 2 * q + 1)
                    for g in pair:
                        k, r = g % nbuf, g // nbuf
                        vector.wait_ge(a_sems[k], 16 * (r + 1))
                        vector.wait_ge(b_sems[k], 16 * (r + 1))
                    base_vi = vi

                    def ph_wait(g_idx, phase):
                        # wait for (phase-1) op of same tile in this pair
                        if phase == 0:
                            if base_vi:
                                vwait(base_vi)
                        else:
                            vwait(base_vi + 2 * (phase - 1) + g_idx + 1)

                    for phase in range(11):
                        for gi, g in enumerate(pair):
                            j = g % RT
                            k = g % nbuf
                            wb = w_bufs[g % 2]
                            ph_wait(gi, phase)
                            if phase == 0:
                                vop(nc.vector.scalar_tensor_tensor(
                                    wb[:, 0, :], a_bufs[k][:], 1.0,
                                    b_bufs[k][:], op0=ALU.mult, op1=ALU.mult))
                            elif phase == 1:
                                vop(nc.vector.scalar_tensor_tensor(
                                    wb[:, 1, :], b_bufs[k][:], 1.0,
                                    b_bufs[k][:], op0=ALU.mult, op1=ALU.mult))
                            elif phase <= 9:
                                w = D >> phase  # 2048 .. 16
                                vop(nc.vector.scalar_tensor_tensor(
                                    wb[:, :, 0:w], wb[:, :, 0:w], 1.0,
                                    wb[:, :, w : 2 * w],
                                    op0=ALU.mult, op1=ALU.add))
                            else:
                                vop(nc.vector.reduce_sum(
                                    accT[:, :, j : j + 1], wb[:, :, 0:16],
                                    axis=X))
            for g in range(NT if variant not in ("tree2", "basep") else 0):
                j = g % RT
                k, r = g % nbuf, g // nbuf
                if variant == "tree":
                    wb = w_bufs[g % 2]
                    vector.wait_ge(a_sems[k], 16 * (r + 1))
                    vector.wait_ge(b_sems[k], 16 * (r + 1))
                    if vi:
                        vwait(vi)
                    vop(
                        nc.vector.scalar_tensor_tensor(
                            wb[:, 0, :], a_bufs[k][:], 1.0, b_bufs[k][:],
                            op0=ALU.mult, op1=ALU.mult,
                        )
                    )
                    vwait(vi)
                    vop(
                        nc.vector.scalar_tensor_tensor(
                            wb[:, 1, :], b_bufs[k][:], 1.0, b_bufs[k][:],
                            op0=ALU.mult, op1=ALU.mult,
                        )
                    )
                    w = D // 2
                    while w >= 16:
                        vwait(vi)
                        vop(
                            nc.vector.scalar_tensor_tensor(
                                wb[:, :, 0:w], wb[:, :, 0:w], 1.0,
                                wb[:, :, w : 2 * w],
                                op0=ALU.mult, op1=ALU.add,
                            )
                        )
                        w //= 2
                    vwait(vi)
                    vop(
                        nc.vector.reduce_sum(
                            accT[:, :, j : j + 1], wb[:, :, 0:16], axis=X
                        )
                    )
                    continue
                vector.wait_ge(a_sems[k], 16 * (r + 1))
                vector.wait_ge(b_sems[k], 16 * (r + 1))
                if vi:
                    vwait(vi)
                if variant == "base":
                    vop(nc.vector.tensor_mul(sd[:], a_bufs[k][:], b_bufs[k][:]))
                    vwait(vi)
                    vop(nc.vector.reduce_sum(num[:, j : j + 1], sd[:], axis=X))
                else:
                    vop(
                        nc.vector.scalar_tensor_tensor(
                            sd[:],
                            a_bufs[k][:],
                            1.0,
                            b_bufs[k][:],
                            op0=ALU.mult,
                            op1=ALU.mult,
                            accum_out=num[:, j : j + 1],
                        )
                    )
                    if variant == "fused2":
                        vwait(vi)
                        vop(
                            nc.vector.scalar_tensor_tensor(
                                sd[:, 0:SD2],
                                b_bufs[k][:, 0:SD2],
                                1.0,
                                b_bufs[k][:, 0:SD2],
                                op0=ALU.mult,
                                op1=ALU.mult,
                                accum_out=n2p[:, j : j + 1],
                            )
                        )
            # ---- epilogue ----
            vector.wait_ge(s_sem, NS_LOOP)  # all n1/n2a ready
            if variant in ("base", "basep"):
                vwait(vi)
                vop(nc.vector.tensor_mul(e_d2[:], n1[:], n2a[:]))
            elif variant in ("tree", "tree2"):
                vwait(vi)
                vop(nc.vector.tensor_mul(e_d2[:], n1[:], accT[:, 1, :]))
            else:
                if variant == "fused3":
                    vector.wait_ge(g_sem, NG_LOOP)  # n2p ready
                vwait(vi)
                vop(nc.vector.tensor_add(e_n2[:], n2a[:], n2p[:]))
                vwait(vi)
                vop(nc.vector.tensor_mul(e_d2[:], n1[:], e_n2[:]))
            vector.wait_ge(s_sem, NS_LOOP + 1)  # den ready
            vwait(vi)
            vop(nc.vector.reciprocal(e_inv[:], e_den[:]))
            vwait(vi)
            num_ap = accT[:, 0, :] if variant in ("tree", "tree2") else num[:]
            vop(nc.vector.tensor_mul(e_cos[:], num_ap, e_inv[:]))
            # dist = 0.5 - 0.5*cos ; de = dist + eps
            vwait(vi)
            vop(
                nc.vector.tensor_scalar(
                    e_dist[:], e_cos[:], -0.5, 0.5, ALU.mult, ALU.add
                )
            )
            vwait(vi)
            vop(nc.vector.tensor_scalar_add(e_de[:], e_dist[:], EPS))
            vector.wait_ge(s_sem, S_TOTAL)  # h ready
            vwait(vi)
            vop(nc.vector.tensor_mul(e_h2[:], e_h[:], e_h[:]))
            vwait(vi)
            vop(nc.vector.tensor_sub(e_dmh[:], e_dist[:], e_h2[:]))
            vector.wait_ge(t_sem, 16)  # t_tile loaded
            vwait(vi)
            vop(nc.vector.tensor_mul(e_tdm[:], t_tile[:], e_dmh[:]))
            vwait(vi)
            vop(nc.vector.tensor_add(e_li[:], e_tdm[:], e_h2[:]))
            vwait(vi)
            vop(nc.vector.reduce_sum(red[:], e_li[:], axis=X))
            assert vi == V_TOTAL, (vi, V_TOTAL)

        def _scalar_body(scalar):
            si = 0

            def sop(inst):
                nonlocal si
                si += 1
                return inst.then_inc(s_sem, 1)

            def swait(idx):
                scalar.wait_ge(s_sem, idx)

            for g in range(NT):
                j = g % RT
                k, r = g % nbuf, g // nbuf
                scalar.wait_ge(a_sems[k], 16 * (r + 1))
                if si:
                    swait(si)
                sop(
                    nc.scalar.activation(
                        sa[:], a_bufs[k][:], AF.Square,
                        accum_out=n1[:, j : j + 1],
                    )
                )
                if variant in ("tree", "tree2"):
                    continue
                scalar.wait_ge(b_sems[k], 16 * (r + 1))
                swait(si)
                if variant == "fused3":
                    sop(
                        nc.scalar.activation(
                            sa[:, pc:D], b_bufs[k][:, pc:D], AF.Square,
                            accum_out=n2a[:, j : j + 1],
                        )
                    )
                elif variant == "fused2":
                    sop(
                        nc.scalar.activation(
                            sa[:, SD2:D], b_bufs[k][:, SD2:D], AF.Square,
                            accum_out=n2a[:, j : j + 1],
                        )
                    )
                else:  # base: full |b|^2 on ACT
                    sop(
                        nc.scalar.activation(
                            sa[:], b_bufs[k][:], AF.Square,
                            accum_out=n2a[:, j : j + 1],
                        )
                    )
            # ---- epilogue ----
            ep_off = 1 if variant in ("base", "basep", "tree", "tree2") else 2
            scalar.wait_ge(v_sem, NV_LOOP + ep_off)  # d2 ready
            swait(si)
            sop(nc.scalar.activation(e_den[:], e_d2[:], AF.Sqrt))
            scalar.wait_ge(v_sem, NV_LOOP + ep_off + 4)  # de ready
            swait(si)
            sop(nc.scalar.activation(e_s[:], e_de[:], AF.Sqrt))
            swait(si)
            sop(
                nc.scalar.activation(
                    e_h[:], e_s[:], AF.Relu, bias=MARGIN, scale=-1.0
                )
            )
            assert si == S_TOTAL, (si, S_TOTAL)

        block.scalar(_scalar_body)

    nc.all_engine_barrier()
    nc.clear_and_free_semaphores(
        [t_sem, *a_sems, *b_sems, out_sem, v_sem, s_sem, g_sem]
    )
    nc.all_engine_barrier()
    return nc


VARIANT = "basep"
IN_DT = "bfloat16"  # input dtype on device; "float32" for full precision


def get_nc_variant(reps, in_dt, variant, nbuf=NBUF, pc=PC):
    key = ("nc", reps, in_dt, variant, nbuf, pc)
    if key not in _CACHE:
        _CACHE[key] = _build_nc(reps, in_dt, variant, nbuf, pc)
    return _CACHE[key]


def get_nc(reps=1, in_dt=None, variant=None):
    return get_nc_variant(reps, in_dt or IN_DT, variant or VARIANT)


def _np_in_dt(in_dt):
    if in_dt == "float32":
        return np.float32
    import ml_dtypes

    return getattr(ml_dtypes, in_dt)


def make_in_maps(output1, output2, target, in_dt=None):
    in_dt = in_dt or IN_DT
    npdt = _np_in_dt(in_dt)
    o1 = np.ascontiguousarray(np.asarray(output1).astype(npdt))
    o2 = np.ascontiguousarray(np.asarray(output2).astype(npdt))
    t = np.asarray(target).astype(np.float32)
    in_maps = []
    for c in range(NCORES):
        sl = slice(c * BS, (c + 1) * BS)
        # t_tile[p, j] = t_core[j*128 + p]
        tcore = np.ascontiguousarray(t[sl].reshape(RT, P).T)
        in_maps.append(
            {
                "output1": np.ascontiguousarray(o1[sl]),
                "output2": np.ascontiguousarray(o2[sl]),
                "target_f32": tcore,
            }
        )
    return in_maps


def kernel(output1, output2, target):
    global LAST_EXEC_TIME_NS
    from concourse.bass_utils import run_bass_kernel_spmd

    nc = get_nc()
    in_maps = make_in_maps(output1, output2, target)
    res = run_bass_kernel_spmd(
        nc, in_maps, core_ids=list(range(NCORES)), trace=TRACE
    )
    LAST_EXEC_TIME_NS = res.exec_time_ns
    total = np.float64(0.0)
    for r in res.results:
        total += r["out"].astype(np.float64).sum()
    mean = 0.5 * total / B
    return np.array(mean, dtype=np.float32)


def _reduce_results(out_shards):
    total = np.float64(0.0)
    for r in out_shards:
        total += np.asarray(r, dtype=np.float64).sum()
    return np.array(0.5 * total / B, dtype=np.float32)


def _make_executable(nc):
    """Replicate run_bass_via_pjrt's sharded executable, returning
    (fn, dev_in_builder, out_avals, n_params). The hook requires the HLO to
    be exactly the bass_exec custom call, so no loops are possible."""
    import jax
    from jax.experimental.shard_map import shard_map
    from jax.sharding import Mesh, NamedSharding, PartitionSpec

    from concourse import mybir
    from concourse.bass2jax import (
        _bass_exec_p,
        install_neuronx_cc_hook,
        partition_id_tensor,
    )

    install_neuronx_cc_hook()
    partition_name = nc.partition_id_tensor.name if nc.partition_id_tensor else None
    in_names, out_names, out_avals, zero_outs = [], [], [], []
    for alloc in nc.m.functions[0].allocations:
        if not isinstance(alloc, mybir.MemoryLocationSet):
            continue
        name = alloc.memorylocations[0].name
        if alloc.kind == "ExternalInput":
            if name != partition_name:
                in_names.append(name)
        elif alloc.kind == "ExternalOutput":
            shape = tuple(alloc.tensor_shape)
            dtype = mybir.dt.np(alloc.dtype)
            out_names.append(name)
            out_avals.append(jax.core.ShapedArray(shape, dtype))
            zero_outs.append(np.zeros(shape, dtype))
    n_params = len(in_names)
    all_names = tuple(
        in_names + out_names + ([partition_name] if partition_name else [])
    )

    def _body(*args):
        operands = list(args)
        operands.append(partition_id_tensor())
        outs = _bass_exec_p.bind(
            *operands,
            out_avals=tuple(out_avals),
            in_names=all_names,
            out_names=tuple(out_names),
            lowering_input_output_aliases=(),
            sim_require_finite=True,
            sim_require_nnan=True,
            nc=nc,
        )
        return tuple(outs)

    devices = jax.devices()[:NCORES]
    mesh = Mesh(np.asarray(devices), ("core",))
    in_specs = (PartitionSpec("core"),) * (n_params + 1)
    out_specs = (PartitionSpec("core"),) * len(out_names)
    fn = jax.jit(
        shard_map(
            _body, mesh=mesh, in_specs=in_specs, out_specs=out_specs,
            check_rep=False,
        ),
        keep_unused=True,
    )
    sharding = NamedSharding(mesh, PartitionSpec("core"))
    return fn, sharding, in_names, out_avals, zero_outs, n_params


def benchmark(output1, output2, target, reps=96, dispatches=(4, 24), nc=None):
    """Measure steady-state device time per full pass over the data.

    The axon relay has ~50-100ms of noisy per-dispatch overhead, so a
    single execution can't be timed. Instead: build a kernel that loops
    the pipeline `reps` times on-device (re-reading the same DRAM), then
    time K back-to-back dispatches for two values of K. The slope is the
    device time per dispatch (~reps passes), immune to the constant
    overhead; divide by reps for per-pass time.
    Returns (result, per_pass_ns, info)."""
    import time

    import jax

    in_maps = make_in_maps(output1, output2, target)
    info = {}

    if nc is None:
        nc = get_nc(reps)
    fn, sharding, in_names, out_avals, zero_outs, n_params = _make_executable(nc)
    per_core = [[np.asarray(m[name]) for name in in_names] for m in in_maps]
    concat_in = [
        np.concatenate([per_core[c][i] for c in range(NCORES)], axis=0)
        for i in range(n_params)
    ]
    dev_in = [jax.device_put(x, sharding) for x in concat_in]
    concat_zero = np.zeros(
        (NCORES * zero_outs[0].shape[0], *zero_outs[0].shape[1:]),
        zero_outs[0].dtype,
    )
    dev_zero = jax.device_put(concat_zero, sharding)

    out = fn(*dev_in, dev_zero)[0]
    out.block_until_ready()  # compile + warmup
    result_arr = np.asarray(out).reshape(NCORES, *out_avals[0].shape)
    result = _reduce_results([result_arr[c] for c in range(NCORES)])

    def timed(k, tries=5):
        best = None
        for _ in range(tries):
            t0 = time.perf_counter()
            last = None
            for _ in range(k):
                last = fn(*dev_in, dev_zero)[0]
            last.block_until_ready()
            dt = time.perf_counter() - t0
            best = dt if best is None else min(best, dt)
        return best

    k1, k2 = dispatches
    # interleave the two dispatch counts so slow-device drift hits both
    # equally instead of biasing the slope
    t1a, t2a = timed(k1, 3), timed(k2, 3)
    t1b, t2b = timed(k1, 3), timed(k2, 3)
    t1, t2 = min(t1a, t1b), min(t2a, t2b)
    per_pass_ns = (t2 - t1) / (k2 - k1) / reps * 1e9
    info["dispatch_times_ms"] = {k1: t1 * 1e3, k2: t2 * 1e3}
    info["reps"] = reps
    _CACHE["last_info"] = info
    return result, per_pass_ns, info
